# revision 3
# baseline (speedup 1.0000x reference)
"""EncoderG (dual-branch TAGConv encoder) as an 8-core SPMD Bass/Tile kernel
for Trainium2.

Sharding: node rows 8-way (1D row partition). Each core c owns output rows
[c*512, (c+1)*512) and holds AT_c = A.T[:, c*512:(c+1)*512] for both
adjacencies. Hop chains y_k = A @ y_{k-1} run in transposed form on the PE —
lhsT = h-chain tile slice (fp8 e4m3, stationary), rhs = AT slice (fp8 e4m3,
moving) — in DoubleRow perf mode (2 fp8 contraction rows per PE cell, 256-row
contraction per matmul), producing y_k^T [feature, local-node] in PSUM, which
is exactly the layout the (f32r) dense layers consume. The node-partition fp8
shard needed for the inter-hop 8-core AllGather is recovered with PE matmuls
against scaled identity matrices (the scale keeps each hop's fp8 operand in
e4m3 range; compensating factors are folded into W1/W2 host-side). The two
branches (G, L) are stage-interleaved so each branch's AllGather+reload hides
under the other branch's hop compute.

Numerics: hop-chain operands fp8 e4m3 (A scaled x4096 into [0,1]; fp32 PSUM
accumulation), dense layers float32r on the exact fp32 PSUM hop results.
BatchNorm (inference) is folded host-side into a per-feature scale/shift
applied by one ScalarE activation (fused with ReLU); conv biases fold into
the BN shift / final bias.

kernel(**inputs) takes the full unsharded inputs and returns the full
[4096, 128] output; per-core outputs are z^T shards assembled host-side.
"""
import numpy as np

N, D, H, Z, KHOPS = 4096, 512, 256, 128, 3
NCORES = 8
R = N // NCORES          # 512 local rows per core
P = 128
MT = R // P              # 4 row tiles per shard
KT = N // P              # 32 contraction tiles
GRP = 4                  # k-blocks per consolidated (DMA-batched) tile
KG = KT // GRP           # 8 big tiles
DT1 = D // P             # 4 conv1 feature tiles
HT = H // P              # 2 hidden feature tiles
EPS = 1e-3               # keras BatchNormalization epsilon

# fp8 chain gains: operand_k = g_k * y_k. g chosen so each fp8 operand sits in
# e4m3 normal range with ~2.5x headroom under the 240 max.
#   chain1 (x-chain): g = [1, N/2, N/2]  -> shard scales [0.5, 1/N]
#   chain2 (h-chain): g = [1, 1, 1]      -> shard scales [1/N, 1/N]
# PSUM_k = N * g_{k-1} * y_k; W folds compensate (see _make_in_maps).
_CACHE = {}


def _build(T=1):
    import concourse.bacc as bacc
    import concourse.tile as tile
    import concourse.mybir as mybir

    F32 = mybir.dt.float32
    F32R = mybir.dt.float32r
    FP8 = mybir.dt.float8e4
    AF = mybir.ActivationFunctionType
    DR = mybir.MatmulPerfMode.DoubleRow

    nc = bacc.Bacc("TRN2", target_bir_lowering=False, debug=False,
                   num_devices=NCORES)

    at = {t: nc.dram_tensor(f"at_{t}", [N, R], FP8, kind="ExternalInput")
          for t in "GL"}
    x_f8 = nc.dram_tensor("x_f8", [N, D], FP8, kind="ExternalInput")
    xt_sh = nc.dram_tensor("xt_sh", [D, R], F32R, kind="ExternalInput")
    w1 = {t: nc.dram_tensor(f"w1_{t}", [D * (KHOPS + 1), H], F32R,
                            kind="ExternalInput") for t in "GL"}
    w2 = {t: nc.dram_tensor(f"w2_{t}", [H * (KHOPS + 1), Z], F32R,
                            kind="ExternalInput") for t in "GL"}
    wm = {t: nc.dram_tensor(f"wm_{t}", [H, Z], F32R, kind="ExternalInput")
          for t in "GL"}
    bn_sc = {t: nc.dram_tensor(f"bn_sc_{t}", [H, 1], F32, kind="ExternalInput")
             for t in "GL"}
    bn_sh = {t: nc.dram_tensor(f"bn_sh_{t}", [H, 1], F32, kind="ExternalInput")
             for t in "GL"}
    zbias = nc.dram_tensor("zbias", [Z, 1], F32, kind="ExternalInput")
    # [I | 0.5*I | I/N] — scaled identities for PE transpose-with-rescale
    ident = nc.dram_tensor("ident", [P, 3 * P], F32R, kind="ExternalInput")
    out_t = nc.dram_tensor("out_t", [Z, R], F32, kind="ExternalOutput")

    RG = [list(range(NCORES))]

    def grp_ap(dram_ap, g, rows_per_grp):
        return dram_ap[g * rows_per_grp:(g + 1) * rows_per_grp, :].rearrange(
            "(b p) d -> p b d", p=P)

    with tile.TileContext(nc) as tc:
        with (
            tc.tile_pool(name="atp", bufs=KG) as atp,
            tc.tile_pool(name="chainp", bufs=KG) as chainp,
            tc.tile_pool(name="wp", bufs=2) as wp,
            tc.tile_pool(name="ysp", bufs=2) as ysp,
            tc.tile_pool(name="h1tp", bufs=2) as h1tp,
            tc.tile_pool(name="smallp", bufs=2) as smallp,
            tc.tile_pool(name="hop_ps", bufs=3, space="PSUM") as hop_ps,
            tc.tile_pool(name="acc1_ps", bufs=4, space="PSUM") as acc1_ps,
            tc.tile_pool(name="acc2_ps", bufs=1, space="PSUM") as acc2_ps,
            tc.tile_pool(name="agin", bufs=2, space="DRAM") as agin,
            tc.tile_pool(name="agout", bufs=2, space="DRAM") as agout,
        ):
            for rep in range(T):
                dma_rr = [0]

                def dma(out_ap, in_ap):
                    # alternate the two HWDGE rings (SP / ACT) for parallelism
                    eng = (nc.sync, nc.scalar)[dma_rr[0] % 2]
                    dma_rr[0] += 1
                    eng.dma_start(out_ap, in_ap)

                ident_t = smallp.tile([P, 3 * P], F32R, name="ident",
                                      tag="ident")
                dma(ident_t[:], ident[:])
                zbias_t = smallp.tile([Z, 1], F32, name="zbias", tag="zb")
                dma(zbias_t[:], zbias[:])

                state = {}
                ACC2_TOTAL = 2 * ((KHOPS + 1) * HT + HT)

                def acc2_mm(lhsT, rhs):
                    nc.tensor.matmul(state["acc2"][:], lhsT, rhs,
                                     start=(state["n"] == 0),
                                     stop=(state["n"] == ACC2_TOTAL - 1))
                    state["n"] += 1

                def transpose_to_fp8(dst_ap, src_ap, sidx, name):
                    # dst = (s * src).T via PE matmul against s*I; DVE copy
                    # converts the f32 PSUM result to the fp8 gather shard.
                    tp = hop_ps.tile([P, P], F32, name=name, tag="hop")
                    nc.tensor.matmul(tp[:], src_ap,
                                     ident_t[:, sidx * P:(sidx + 1) * P],
                                     start=True, stop=True)
                    nc.vector.tensor_copy(dst_ap, tp[:])

                def allgather(shard_big, width, tag, branch_tag):
                    bounce_in = agin.tile([R, width], FP8, name=f"agi_{tag}",
                                          tag="agin")
                    dma(bounce_in[:].rearrange("(b p) d -> p b d", p=P),
                        shard_big[:].rearrange("p (b d) -> p b d", b=MT))
                    bounce_out = agout.tile([N, width], FP8, name=f"ago_{tag}",
                                            tag="agout", addr_space="Shared")
                    nc.gpsimd.collective_compute(
                        "AllGather", mybir.AluOpType.bypass, replica_groups=RG,
                        ins=[bounce_in.opt()], outs=[bounce_out.opt()])
                    tiles = []
                    for g in range(KG):
                        t = chainp.tile([P, GRP, width], FP8,
                                        name=f"h_{tag}_{g}",
                                        tag=f"chain{branch_tag}")
                        dma(t[:], grp_ap(bounce_out, g, GRP * P))
                        tiles.append(t)
                    return tiles

                def hop_matmuls(h_tiles, at_t, width, name):
                    ndt = width // P
                    yts = ysp.tile([P, ndt * R], F32R, name=f"{name}_yts",
                                   tag="ys")
                    for d0 in range(0, ndt, 2):
                        dts = range(d0, min(d0 + 2, ndt))
                        yt_ps = {dt: hop_ps.tile([P, R], F32,
                                                 name=f"{name}_ps{dt}",
                                                 tag="hop") for dt in dts}
                        for u in range(KT // 2):
                            g, b0 = u // 2, 2 * (u % 2)
                            rhs = at_t[g][:, b0:b0 + 2, :]
                            for dt in dts:
                                lhsT = h_tiles[g][:, b0:b0 + 2,
                                                  dt * P:(dt + 1) * P]
                                nc.tensor.matmul(yt_ps[dt][:], lhsT, rhs,
                                                 start=(u == 0),
                                                 stop=(u == KT // 2 - 1),
                                                 perf_mode=DR)
                        for dt in dts:
                            nc.vector.tensor_copy(yts[:, dt * R:(dt + 1) * R],
                                                  yt_ps[dt][:])
                    return yts

                def to_node_shard(yts, width, sidx, name):
                    ndt = width // P
                    shard = ysp.tile([P, MT * width], FP8, name=f"{name}_sh",
                                     tag="ys")
                    for m in range(MT):
                        for dt in range(ndt):
                            transpose_to_fp8(
                                shard[:, m * width + dt * P:
                                      m * width + (dt + 1) * P],
                                yts[:, dt * R + m * P:dt * R + (m + 1) * P],
                                sidx, f"{name}_tp{m}_{dt}")
                    return shard

                def branch(tag):
                    if tag == "G":
                        at_t = state["atG_tiles"]
                    else:
                        at_t = []
                        for g in range(KG):
                            t = atp.tile([P, GRP, R], FP8, name=f"at{tag}_{g}",
                                         tag=f"at{tag}")
                            dma(t[:], grp_ap(at[tag], g, GRP * P))
                            at_t.append(t)
                    h_tiles = state["x_tiles"]
                    w1a_t = wp.tile([P, DT1 * H], F32R,
                                    name=f"w1{tag}" + "a", tag="w1a")
                    dma(w1a_t[:].rearrange("p (b d) -> p b d", p=P, d=H),
                        w1[tag][:DT1 * P, :].rearrange("(b p) d -> p b d", p=P))
                    w1b_t = wp.tile([P, KHOPS * DT1 * H], F32R,
                                    name=f"w1{tag}" + "b", tag="w1b")
                    dma(w1b_t[:].rearrange("p (b d) -> p b d", p=P, d=H),
                        w1[tag][DT1 * P:, :].rearrange("(b p) d -> p b d", p=P))
                    w2_t = wp.tile([P, (KHOPS + 1) * HT * Z], F32R,
                                   name=f"w2{tag}", tag="w2")
                    dma(w2_t[:].rearrange("p (b d) -> p b d", p=P, d=Z),
                        w2[tag][:].rearrange("(b p) d -> p b d", p=P))
                    wm_t = wp.tile([P, HT * Z], F32R, name=f"wm{tag}",
                                   tag="wm")
                    dma(wm_t[:].rearrange("p (b d) -> p b d", p=P, d=Z),
                        wm[tag][:].rearrange("(b p) d -> p b d", p=P))
                    bn_sc_t = smallp.tile([P, HT], F32, name=f"bnsc{tag}",
                                          tag="bn1")
                    dma(bn_sc_t[:].rearrange("p (b d) -> p b d", p=P, d=1),
                        bn_sc[tag][:].rearrange("(b p) d -> p b d", p=P))
                    bn_sh_t = smallp.tile([P, HT], F32, name=f"bnsh{tag}",
                                          tag="bn2")
                    dma(bn_sh_t[:].rearrange("p (b d) -> p b d", p=P, d=1),
                        bn_sh[tag][:].rearrange("(b p) d -> p b d", p=P))
                    xt_t = state["xt_t"]

                    def w1_slice(khop, dt, hf):
                        if khop == 0:
                            base = dt * H + hf * P
                            return w1a_t[:, base:base + P]
                        base = ((khop - 1) * DT1 + dt) * H + hf * P
                        return w1b_t[:, base:base + P]

                    def w2_slice(khop, dt):
                        base = (khop * HT + dt) * Z
                        return w2_t[:, base:base + Z]

                    acc1 = [acc1_ps.tile([P, R], F32, name=f"acc1{tag}_{hf}",
                                         tag="acc1") for hf in range(HT)]
                    n1 = DT1 * (KHOPS + 1)
                    cnt1 = [0, 0]

                    def dense1_mm(hf, lhsT, rhs):
                        nc.tensor.matmul(acc1[hf][:], lhsT, rhs,
                                         start=(cnt1[hf] == 0),
                                         stop=(cnt1[hf] == n1 - 1))
                        cnt1[hf] += 1

                    # shard rescale (identity block) per hop: chain1 keeps the
                    # operand near N*y (x-chain decays fast); chain2 descales
                    # by 1/N every hop (h-chain values are O(0.1-1)).
                    sidx1 = {1: 1, 2: 2}
                    for khop in range(1, KHOPS + 1):
                        yts = hop_matmuls(h_tiles, at_t, D, f"y{tag}{khop}")
                        if khop < KHOPS:
                            shard = to_node_shard(yts, D, sidx1[khop],
                                                  f"y{tag}{khop}")
                            h_tiles = allgather(shard, D, f"{tag}1_{khop}",
                                                tag)
                        for dt in range(DT1):
                            for hf in range(HT):
                                dense1_mm(hf, w1_slice(khop, dt, hf),
                                          yts[:, dt * R:(dt + 1) * R])
                        if khop == 1:
                            for dt in range(DT1):
                                for hf in range(HT):
                                    dense1_mm(hf, w1_slice(0, dt, hf),
                                              xt_t[:, dt * R:(dt + 1) * R])
                        if khop < KHOPS:
                            yield

                    h1t = []
                    for hf in range(HT):
                        t = h1tp.tile([P, R], F32R, name=f"h1t{tag}_{hf}",
                                      tag="h1t")
                        nc.scalar.activation(t[:], acc1[hf][:], AF.Relu,
                                             bias=bn_sh_t[:, hf:hf + 1],
                                             scale=bn_sc_t[:, hf:hf + 1])
                        h1t.append(t)

                    for dt in range(HT):
                        acc2_mm(w2_slice(0, dt), h1t[dt][:])
                    for dt in range(HT):
                        acc2_mm(wm_t[:, dt * Z:(dt + 1) * Z], h1t[dt][:])

                    h1ts = ysp.tile([P, HT * R], F32R, name=f"h1ts{tag}",
                                    tag="ys")
                    for hf in range(HT):
                        nc.vector.tensor_copy(h1ts[:, hf * R:(hf + 1) * R],
                                              h1t[hf][:])
                    shard = to_node_shard(h1ts, H, 0, f"h1{tag}")
                    h_tiles = allgather(shard, H, f"{tag}2_0", tag)
                    yield

                    for khop in range(1, KHOPS + 1):
                        yts = hop_matmuls(h_tiles, at_t, H, f"z{tag}{khop}")
                        for dt in range(HT):
                            acc2_mm(w2_slice(khop, dt),
                                    yts[:, dt * R:(dt + 1) * R])
                        if khop < KHOPS:
                            shard = to_node_shard(yts, H, 2, f"z{tag}{khop}")
                            h_tiles = allgather(shard, H, f"{tag}2_{khop}",
                                                tag)
                            yield

                state["acc2"] = acc2_ps.tile([P, R], F32, name="acc2",
                                             tag="acc2")
                state["n"] = 0
                x_tiles = []
                atG_tiles = []
                for g in range(KG):
                    a = atp.tile([P, GRP, R], FP8, name=f"atG_{g}", tag="atG")
                    dma(a[:], grp_ap(at["G"], g, GRP * P))
                    atG_tiles.append(a)
                    t = chainp.tile([P, GRP, D], FP8, name=f"x_{g}",
                                    tag="chainG")
                    dma(t[:], grp_ap(x_f8, g, GRP * P))
                    x_tiles.append(t)
                state["atG_tiles"] = atG_tiles
                state["x_tiles"] = x_tiles
                xt_t = ysp.tile([P, DT1 * R], F32R, name="xt", tag="xt",
                                bufs=1)
                dma(xt_t[:].rearrange("p (b d) -> p b d", b=DT1),
                    xt_sh[:].rearrange("(b p) d -> p b d", p=P))
                state["xt_t"] = xt_t

                gens = [branch("G"), branch("L")]
                done = [False, False]
                while not all(done):
                    for i, g in enumerate(gens):
                        if not done[i]:
                            try:
                                next(g)
                            except StopIteration:
                                done[i] = True

                out_sb = ysp.tile([Z, R], F32, name="out_sb", tag="ys")
                nc.vector.tensor_scalar_add(out_sb[:], state["acc2"][:],
                                            zbias_t[:])
                dma(out_t[:], out_sb[:])

    nc.compile()
    return nc


def _make_in_maps(inputs):
    import ml_dtypes
    f8 = ml_dtypes.float8_e4m3
    x = np.asarray(inputs["x"], np.float32)
    at_full = {t: np.ascontiguousarray(
        (np.asarray(inputs[f"A_{t}"], np.float32).T * N).astype(f8))
        for t in "GL"}
    # fp8 chain scale bookkeeping: PSUM_k = N * g_{k-1} * y_k with
    #   chain1 g = [1, N/2, N/2]   chain2 g = [1, 1, 1]
    fold1 = [1.0, 1.0 / N, 2.0 / (N * N), 2.0 / (N * N)]
    fold2 = [1.0, 1.0 / N, 1.0 / N, 1.0 / N]
    prep = {}
    for t in "GL":
        g = np.asarray(inputs[f"gamma_{t}"], np.float32)
        b = np.asarray(inputs[f"beta_{t}"], np.float32)
        mu = np.asarray(inputs[f"mean_{t}"], np.float32)
        v = np.asarray(inputs[f"var_{t}"], np.float32)
        b1 = np.asarray(inputs[f"b1_{t}"], np.float32)
        sc = g / np.sqrt(v + EPS)
        sh = (b1 - mu) * sc + b
        prep[f"bn_sc_{t}"] = np.ascontiguousarray(sc.reshape(H, 1))
        prep[f"bn_sh_{t}"] = np.ascontiguousarray(sh.reshape(H, 1))
        w1s = np.array(inputs[f"W1_{t}"], np.float32)
        for k in range(KHOPS + 1):
            w1s[k * D:(k + 1) * D] *= fold1[k]
        prep[f"w1_{t}"] = np.ascontiguousarray(w1s)
        w2s = np.array(inputs[f"W2_{t}"], np.float32)
        for k in range(KHOPS + 1):
            w2s[k * H:(k + 1) * H] *= fold2[k]
        prep[f"w2_{t}"] = np.ascontiguousarray(w2s)
        prep[f"wm_{t}"] = np.ascontiguousarray(inputs[f"Wm_{t}"], np.float32)
    zb = sum(np.asarray(inputs[f"b2_{t}"], np.float32) +
             np.asarray(inputs[f"bm_{t}"], np.float32) for t in "GL")
    prep["zbias"] = np.ascontiguousarray(zb.reshape(Z, 1))
    eye = np.eye(P, dtype=np.float32)
    prep["ident"] = np.ascontiguousarray(
        np.concatenate([eye, 0.5 * eye, eye / N], axis=1))
    prep["x_f8"] = np.ascontiguousarray(x.astype(f8))
    in_maps = []
    for c in range(NCORES):
        sl = slice(c * R, (c + 1) * R)
        m = dict(prep)
        m["xt_sh"] = np.ascontiguousarray(x[sl].T)
        for t in "GL":
            m[f"at_{t}"] = np.ascontiguousarray(at_full[t][:, sl])
        in_maps.append(m)
    return in_maps


def _get_nc():
    if "nc" not in _CACHE:
        _CACHE["nc"] = _build()
    return _CACHE["nc"]


def kernel(**inputs) -> np.ndarray:
    from concourse.bass_utils import run_bass_kernel_spmd

    nc = _get_nc()
    in_maps = _make_in_maps(inputs)
    res = run_bass_kernel_spmd(nc, in_maps, list(range(NCORES)))
    out = np.empty((N, Z), np.float32)
    for c in range(NCORES):
        out[c * R:(c + 1) * R, :] = res.results[c]["out_t"].T
    return out


# revision 9
# speedup vs baseline: 2.3271x; 2.3271x over previous
"""EncoderG (dual-branch TAGConv encoder) as an 8-core SPMD Bass/Tile kernel
for Trainium2.

Sharding: node rows 8-way (1D row partition). Each core c owns output rows
[c*512, (c+1)*512) and holds AT_c = A.T[:, c*512:(c+1)*512] for both
adjacencies (fp8 e4m3, pre-scaled by N so entries sit in [0,1]).

Algorithm: TAGConv is reassociated into Horner form —
    concat([x, Ax, ..., A^K x]) @ W1 = u_0 + A(u_1 + A(u_2 + A u_3)),
with u_k = x @ W1_k — so every SpMM hop runs at the OUTPUT width (H=256 for
conv1, Z=128 for conv2) instead of the input width. This halves the dominant
hop FLOPs vs the direct form. Each Horner stage s_k = u_k + A s_{k+1} is one
PSUM accumulation group: f32r matmuls for u_k^T (W1_k as stationary, x^T as
moving) followed by fp8 DoubleRow matmuls for the A-hop (s_{k+1} chain tiles
stationary, AT moving, 256-row contraction per instruction), producing
s_k^T [feature, local-node] in PSUM. The node-partition fp8 shard needed for
the inter-hop 8-core AllGather is recovered with PE matmuls against I/N
(descaling the xN weight fold for free). The two branches (G, L) are
stage-interleaved so each branch's AllGather+reload hides under the other
branch's compute.

Numerics: hop operands fp8 e4m3 (f32 PSUM accumulation), dense u_k/v_k/Wm
matmuls f32r with weights folded xN host-side (descale 1/N applied at the
shard quantization, the BN scale, and the final output copy). BatchNorm
(inference) is folded host-side into a per-feature scale/shift applied by one
ScalarE activation (fused with ReLU); conv biases fold into the BN shift /
final bias.

kernel(**inputs) takes the full unsharded inputs and returns the full
[4096, 128] output; per-core outputs are z^T shards assembled host-side.
"""
import numpy as np

N, D, H, Z, KHOPS = 4096, 512, 256, 128, 3
NCORES = 8
R = N // NCORES          # 512 local rows per core
P = 128
MT = R // P              # 4 row tiles per shard
KT = N // P              # 32 contraction tiles
GRP = 4                  # k-blocks per consolidated (DMA-batched) tile
KG = KT // GRP           # 8 big tiles
DT1 = D // P             # 4 conv1 feature tiles
HT = H // P              # 2 hidden feature tiles
EPS = 1e-3               # keras BatchNormalization epsilon

_CACHE = {}


def _build(T=1):
    import concourse.bacc as bacc
    import concourse.tile as tile
    import concourse.mybir as mybir

    F32 = mybir.dt.float32
    F32R = mybir.dt.float32r
    FP8 = mybir.dt.float8e4
    AF = mybir.ActivationFunctionType
    DR = mybir.MatmulPerfMode.DoubleRow

    nc = bacc.Bacc("TRN2", target_bir_lowering=False, debug=False,
                   num_devices=NCORES)

    at = {t: nc.dram_tensor(f"at_{t}", [N, R], FP8, kind="ExternalInput")
          for t in "GL"}
    xt_sh = nc.dram_tensor("xt_sh", [D, R], F32R, kind="ExternalInput")
    w1 = {t: nc.dram_tensor(f"w1_{t}", [D * (KHOPS + 1), H], F32R,
                            kind="ExternalInput") for t in "GL"}
    w2 = {t: nc.dram_tensor(f"w2_{t}", [H * (KHOPS + 1), Z], F32R,
                            kind="ExternalInput") for t in "GL"}
    wm = {t: nc.dram_tensor(f"wm_{t}", [H, Z], F32R, kind="ExternalInput")
          for t in "GL"}
    bn_sc = {t: nc.dram_tensor(f"bn_sc_{t}", [H, 1], F32, kind="ExternalInput")
             for t in "GL"}
    bn_sh = {t: nc.dram_tensor(f"bn_sh_{t}", [H, 1], F32, kind="ExternalInput")
             for t in "GL"}
    zbias = nc.dram_tensor("zbias", [Z, 1], F32, kind="ExternalInput")
    ident = nc.dram_tensor("ident", [P, P], F32R,
                           kind="ExternalInput")  # I/N
    out_t = nc.dram_tensor("out_t", [Z, R], F32, kind="ExternalOutput")

    RG = [list(range(NCORES))]

    def grp_ap(dram_ap, g, rows_per_grp):
        return dram_ap[g * rows_per_grp:(g + 1) * rows_per_grp, :].rearrange(
            "(b p) d -> p b d", p=P)

    with tile.TileContext(nc) as tc:
        with (
            tc.tile_pool(name="atp", bufs=KG) as atp,
            tc.tile_pool(name="chainp", bufs=KG) as chainp,
            tc.tile_pool(name="wp", bufs=2) as wp,
            tc.tile_pool(name="ysp", bufs=3) as ysp,
            tc.tile_pool(name="h1tp", bufs=4) as h1tp,
            tc.tile_pool(name="smallp", bufs=2) as smallp,
            tc.tile_pool(name="hop_ps", bufs=5, space="PSUM") as hop_ps,
            tc.tile_pool(name="tp_ps", bufs=2, space="PSUM") as tp_ps,
            tc.tile_pool(name="acc2_ps", bufs=1, space="PSUM") as acc2_ps,
            tc.tile_pool(name="agin", bufs=2, space="DRAM") as agin,
            tc.tile_pool(name="agout", bufs=2, space="DRAM") as agout,
        ):
            for rep in range(T):
                dma_rr = [0]

                def dma(out_ap, in_ap):
                    # alternate the two HWDGE rings (SP / ACT) for parallelism
                    eng = (nc.sync, nc.scalar)[dma_rr[0] % 2]
                    dma_rr[0] += 1
                    eng.dma_start(out_ap, in_ap)

                ident_t = smallp.tile([P, P], F32R, name="ident", tag="ident")
                dma(ident_t[:], ident[:])
                zbias_t = smallp.tile([Z, 1], F32, name="zbias", tag="zb")
                dma(zbias_t[:], zbias[:])

                state = {}
                ACC2_TOTAL = 2 * (HT + KT // 2 + HT)

                def acc2_mm(lhsT, rhs, perf_mode=None):
                    nc.tensor.matmul(state["acc2"][:], lhsT, rhs,
                                     start=(state["n"] == 0),
                                     stop=(state["n"] == ACC2_TOTAL - 1),
                                     perf_mode=perf_mode)
                    state["n"] += 1

                def transpose_fp8(dst_ap, src_ap, name):
                    # dst = (src / N).T via PE matmul against I/N; DVE copy
                    # converts the f32 PSUM result to the fp8 gather shard.
                    tp = tp_ps.tile([P, P], F32, name=name, tag="tp")
                    nc.tensor.matmul(tp[:], src_ap, ident_t[:],
                                     start=True, stop=True)
                    nc.vector.tensor_copy(dst_ap, tp[:])

                def allgather(shard_big, width, tag, branch_tag):
                    bounce_in = agin.tile([R, width], FP8, name=f"agi_{tag}",
                                          tag="agin")
                    dma(bounce_in[:].rearrange("(b p) d -> p b d", p=P),
                        shard_big[:].rearrange("p (b d) -> p b d", b=MT))
                    bounce_out = agout.tile([N, width], FP8, name=f"ago_{tag}",
                                            tag="agout", addr_space="Shared")
                    nc.gpsimd.collective_compute(
                        "AllGather", mybir.AluOpType.bypass, replica_groups=RG,
                        ins=[bounce_in.opt()], outs=[bounce_out.opt()])
                    tiles = []
                    for g in range(KG):
                        t = chainp.tile([P, GRP, width], FP8,
                                        name=f"s_{tag}_{g}",
                                        tag=f"chain{branch_tag}")
                        dma(t[:], grp_ap(bounce_out, g, GRP * P))
                        tiles.append(t)
                    return tiles

                def hop_drmm(ps_list, s_tiles, at_t, width, stop_at_end):
                    # ps_list: one [P, R] PSUM per 128-wide feature slice.
                    ndt = width // P
                    for u in range(KT // 2):
                        g, b0 = u // 2, 2 * (u % 2)
                        rhs = at_t[g][:, b0:b0 + 2, :]
                        for dt in range(ndt):
                            nc.tensor.matmul(
                                ps_list[dt][:],
                                s_tiles[g][:, b0:b0 + 2, dt * P:(dt + 1) * P],
                                rhs, start=False,
                                stop=(stop_at_end and u == KT // 2 - 1),
                                perf_mode=DR)

                def to_shard_and_gather(ps_list, width, tag, branch_tag):
                    # PSUM stages -> SBUF f32r -> fp8 node-partition shard
                    # (transposed + descaled by 1/N on PE) -> AllGather.
                    ndt = width // P
                    st = ysp.tile([P, ndt * R], F32R, name=f"st_{tag}",
                                  tag=f"ys{branch_tag}")
                    for dt in range(ndt):
                        nc.vector.tensor_copy(st[:, dt * R:(dt + 1) * R],
                                              ps_list[dt][:])
                    shard = ysp.tile([P, MT * width], FP8, name=f"sh_{tag}",
                                     tag=f"ys{branch_tag}")
                    for m in range(MT):
                        for dt in range(ndt):
                            transpose_fp8(
                                shard[:, m * width + dt * P:
                                      m * width + (dt + 1) * P],
                                st[:, dt * R + m * P:dt * R + (m + 1) * P],
                                f"tp_{tag}_{m}_{dt}")
                    return allgather(shard, width, tag, branch_tag)

                def branch(tag):
                    if tag == "L":
                        at_t = []
                        for g in range(KG):
                            t = atp.tile([P, GRP, R], FP8, name=f"at{tag}_{g}",
                                         tag=f"at{tag}")
                            dma(t[:], grp_ap(at[tag], g, GRP * P))
                            at_t.append(t)
                    else:
                        at_t = state["atG_tiles"]
                    w1_t = wp.tile([P, (KHOPS + 1) * DT1 * H], F32R,
                                   name=f"w1{tag}", tag="w1")
                    dma(w1_t[:].rearrange("p (b d) -> p b d", p=P, d=H),
                        w1[tag][:].rearrange("(b p) d -> p b d", p=P))
                    w2_t = wp.tile([P, (KHOPS + 1) * HT * Z], F32R,
                                   name=f"w2{tag}", tag="w2")
                    dma(w2_t[:].rearrange("p (b d) -> p b d", p=P, d=Z),
                        w2[tag][:].rearrange("(b p) d -> p b d", p=P))
                    wm_t = wp.tile([P, HT * Z], F32R, name=f"wm{tag}",
                                   tag="wm")
                    dma(wm_t[:].rearrange("p (b d) -> p b d", p=P, d=Z),
                        wm[tag][:].rearrange("(b p) d -> p b d", p=P))
                    bn_sc_t = smallp.tile([P, HT], F32, name=f"bnsc{tag}",
                                          tag="bn1")
                    dma(bn_sc_t[:].rearrange("p (b d) -> p b d", p=P, d=1),
                        bn_sc[tag][:].rearrange("(b p) d -> p b d", p=P))
                    bn_sh_t = smallp.tile([P, HT], F32, name=f"bnsh{tag}",
                                          tag="bn2")
                    dma(bn_sh_t[:].rearrange("p (b d) -> p b d", p=P, d=1),
                        bn_sh[tag][:].rearrange("(b p) d -> p b d", p=P))
                    xt_t = state["xt_t"]
                    yield

                    def w1_slice(khop, dt, hf):
                        base = (khop * DT1 + dt) * H + hf * P
                        return w1_t[:, base:base + P]

                    def w2_slice(khop, dt):
                        base = (khop * HT + dt) * Z
                        return w2_t[:, base:base + Z]

                    # ---- conv1 Horner: s_k = u_k + A s_{k+1}, k = K..1 ----
                    s_tiles = None
                    for k in range(KHOPS, 0, -1):
                        ps = [hop_ps.tile([P, R], F32, name=f"s{tag}{k}_{hf}",
                                          tag="hop") for hf in range(HT)]
                        last_u = (k == KHOPS)
                        for hf in range(HT):
                            for dt in range(DT1):
                                nc.tensor.matmul(
                                    ps[hf][:], w1_slice(k, dt, hf),
                                    xt_t[:, dt * R:(dt + 1) * R],
                                    start=(dt == 0),
                                    stop=(last_u and dt == DT1 - 1))
                        yield
                        if k < KHOPS:
                            hop_drmm(ps, s_tiles, at_t, H, True)
                        s_tiles = to_shard_and_gather(ps, H, f"{tag}1_{k}",
                                                      tag)
                        yield

                    # ---- h = relu(BN(u_0 + A s_1)) ----
                    acc1 = [hop_ps.tile([P, R], F32, name=f"acc1{tag}_{hf}",
                                        tag="hop") for hf in range(HT)]
                    for hf in range(HT):
                        for dt in range(DT1):
                            nc.tensor.matmul(
                                acc1[hf][:], w1_slice(0, dt, hf),
                                xt_t[:, dt * R:(dt + 1) * R],
                                start=(dt == 0), stop=False)
                    yield
                    hop_drmm(acc1, s_tiles, at_t, H, True)
                    h1t = []
                    for hf in range(HT):
                        t = h1tp.tile([P, R], F32R, name=f"h1t{tag}_{hf}",
                                      tag="h1t")
                        nc.scalar.activation(t[:], acc1[hf][:], AF.Relu,
                                             bias=bn_sh_t[:, hf:hf + 1],
                                             scale=bn_sc_t[:, hf:hf + 1])
                        h1t.append(t)
                    yield

                    # ---- conv2 Horner: s'_k = v_k + A s'_{k+1}, k = K..1 ----
                    s_tiles = None
                    for k in range(KHOPS, 0, -1):
                        ps = [hop_ps.tile([P, R], F32, name=f"z{tag}{k}",
                                          tag="hop")]
                        last_v = (k == KHOPS)
                        for dt in range(HT):
                            nc.tensor.matmul(
                                ps[0][:], w2_slice(k, dt), h1t[dt][:],
                                start=(dt == 0),
                                stop=(last_v and dt == HT - 1))
                        yield
                        if k < KHOPS:
                            hop_drmm(ps, s_tiles, at_t, Z, True)
                        s_tiles = to_shard_and_gather(ps, Z, f"{tag}2_{k}",
                                                      tag)
                        yield

                    # ---- z = v_0 + A s'_1 + h @ (N Wm), into shared acc2 ----
                    for dt in range(HT):
                        acc2_mm(w2_slice(0, dt), h1t[dt][:])
                    for u in range(KT // 2):
                        g, b0 = u // 2, 2 * (u % 2)
                        acc2_mm(s_tiles[g][:, b0:b0 + 2, :],
                                at_t[g][:, b0:b0 + 2, :], perf_mode=DR)
                    for dt in range(HT):
                        acc2_mm(wm_t[:, dt * Z:(dt + 1) * Z], h1t[dt][:])

                state["acc2"] = acc2_ps.tile([P, R], F32, name="acc2",
                                             tag="acc2")
                state["n"] = 0
                at_t = []
                for g in range(KG):
                    a = atp.tile([P, GRP, R], FP8, name=f"atG_{g}", tag="atG")
                    dma(a[:], grp_ap(at["G"], g, GRP * P))
                    at_t.append(a)
                state["atG_tiles"] = at_t
                xt_t = ysp.tile([P, DT1 * R], F32R, name="xt", tag="xt",
                                bufs=1)
                dma(xt_t[:].rearrange("p (b d) -> p b d", b=DT1),
                    xt_sh[:].rearrange("(b p) d -> p b d", p=P))
                state["xt_t"] = xt_t

                gens = [branch("G"), branch("L")]
                done = [False, False]
                while not all(done):
                    for i, g in enumerate(gens):
                        if not done[i]:
                            try:
                                next(g)
                            except StopIteration:
                                done[i] = True

                out_sb = ysp.tile([Z, R], F32, name="out_sb", tag="out")
                nc.scalar.activation(out_sb[:], state["acc2"][:], AF.Identity,
                                     bias=zbias_t[:], scale=1.0 / N)
                dma(out_t[:], out_sb[:])

    nc.compile()
    return nc


def _make_in_maps(inputs):
    import ml_dtypes
    f8 = ml_dtypes.float8_e4m3
    x = np.asarray(inputs["x"], np.float32)
    at_full = {t: np.ascontiguousarray(
        (np.asarray(inputs[f"A_{t}"], np.float32).T * N).astype(f8))
        for t in "GL"}
    prep = {}
    for t in "GL":
        g = np.asarray(inputs[f"gamma_{t}"], np.float32)
        b = np.asarray(inputs[f"beta_{t}"], np.float32)
        mu = np.asarray(inputs[f"mean_{t}"], np.float32)
        v = np.asarray(inputs[f"var_{t}"], np.float32)
        b1 = np.asarray(inputs[f"b1_{t}"], np.float32)
        sc = g / np.sqrt(v + EPS)
        sh = (b1 - mu) * sc + b
        # acc1 PSUM carries N * conv1 — fold the descale into the BN scale
        prep[f"bn_sc_{t}"] = np.ascontiguousarray((sc / N).reshape(H, 1))
        prep[f"bn_sh_{t}"] = np.ascontiguousarray(sh.reshape(H, 1))
        prep[f"w1_{t}"] = np.ascontiguousarray(
            np.asarray(inputs[f"W1_{t}"], np.float32) * N)
        prep[f"w2_{t}"] = np.ascontiguousarray(
            np.asarray(inputs[f"W2_{t}"], np.float32) * N)
        prep[f"wm_{t}"] = np.ascontiguousarray(
            np.asarray(inputs[f"Wm_{t}"], np.float32) * N)
    zb = sum(np.asarray(inputs[f"b2_{t}"], np.float32) +
             np.asarray(inputs[f"bm_{t}"], np.float32) for t in "GL")
    prep["zbias"] = np.ascontiguousarray(zb.reshape(Z, 1))
    prep["ident"] = np.ascontiguousarray(np.eye(P, dtype=np.float32) / N)
    in_maps = []
    for c in range(NCORES):
        sl = slice(c * R, (c + 1) * R)
        m = dict(prep)
        m["xt_sh"] = np.ascontiguousarray(x[sl].T)
        for t in "GL":
            m[f"at_{t}"] = np.ascontiguousarray(at_full[t][:, sl])
        in_maps.append(m)
    return in_maps


def _get_nc():
    if "nc" not in _CACHE:
        _CACHE["nc"] = _build()
    return _CACHE["nc"]


def kernel(**inputs) -> np.ndarray:
    from concourse.bass_utils import run_bass_kernel_spmd

    nc = _get_nc()
    in_maps = _make_in_maps(inputs)
    res = run_bass_kernel_spmd(nc, in_maps, list(range(NCORES)))
    out = np.empty((N, Z), np.float32)
    for c in range(NCORES):
        out[c * R:(c + 1) * R, :] = res.results[c]["out_t"].T
    return out


# revision 19
# speedup vs baseline: 3.4859x; 1.4979x over previous
"""EncoderG (dual-branch TAGConv encoder) as an 8-core SPMD Bass/Tile kernel
for Trainium2.

Sharding: node rows 8-way (1D row partition). Each core c owns output rows
[c*512, (c+1)*512) and holds AT_c = A.T[:, c*512:(c+1)*512] for both
adjacencies (fp8 e4m3, pre-scaled by N so entries sit in [0,1]).

Algorithm: TAGConv is reassociated into Horner form —
    concat([x, Ax, ..., A^K x]) @ W1 = u_0 + A(u_1 + A(u_2 + A u_3)),
with u_k = x @ W1_k — so every SpMM hop runs at the OUTPUT width (H=256 for
conv1, Z=128 for conv2) instead of the input width. This halves the dominant
hop FLOPs vs the direct form. Each Horner stage s_k = u_k + A s_{k+1} is one
PSUM accumulation group: f32r matmuls for u_k^T (W1_k as stationary, x^T as
moving) followed by fp8 DoubleRow matmuls for the A-hop (s_{k+1} chain tiles
stationary, AT moving, 256-row contraction per instruction), producing
s_k^T [feature, local-node] in PSUM. The node-partition fp8 shard needed for
the inter-hop 8-core AllGather is recovered with PE matmuls against I/N
(descaling the xN weight fold for free). The two branches (G, L) are
stage-interleaved so each branch's AllGather+reload hides under the other
branch's compute.

Numerics: hop operands fp8 e4m3 (f32 PSUM accumulation), dense u_k/v_k/Wm
matmuls f32r with weights folded xN host-side (descale 1/N applied at the
shard quantization, the BN scale, and the final output copy). BatchNorm
(inference) is folded host-side into a per-feature scale/shift applied by one
ScalarE activation (fused with ReLU); conv biases fold into the BN shift /
final bias.

kernel(**inputs) takes the full unsharded inputs and returns the full
[4096, 128] output; per-core outputs are z^T shards assembled host-side.
"""
import numpy as np

N, D, H, Z, KHOPS = 4096, 512, 256, 128, 3
NCORES = 8
R = N // NCORES          # 512 local rows per core
P = 128
MT = R // P              # 4 row tiles per shard
KT = N // P              # 32 contraction tiles
GRP = 4                  # k-blocks per consolidated (DMA-batched) tile
KG = KT // GRP           # 8 big tiles
DT1 = D // P             # 4 conv1 feature tiles
HT = H // P              # 2 hidden feature tiles
EPS = 1e-3               # keras BatchNormalization epsilon

_CACHE = {}


def _build(T=1):
    import concourse.bacc as bacc
    import concourse.tile as tile
    import concourse.mybir as mybir

    F32 = mybir.dt.float32
    F32R = mybir.dt.float32r
    FP8 = mybir.dt.float8e4
    AF = mybir.ActivationFunctionType
    DR = mybir.MatmulPerfMode.DoubleRow

    nc = bacc.Bacc("TRN2", target_bir_lowering=False, debug=False,
                   num_devices=NCORES)

    at = {t: nc.dram_tensor(f"at_{t}", [N, R], FP8, kind="ExternalInput")
          for t in "GL"}
    xt_sh = nc.dram_tensor("xt_sh", [D, R], F32R, kind="ExternalInput")
    w1 = {t: nc.dram_tensor(f"w1_{t}", [D * (KHOPS + 1), H], F32R,
                            kind="ExternalInput") for t in "GL"}
    w2 = {t: nc.dram_tensor(f"w2_{t}", [H * (KHOPS + 1), Z], F32R,
                            kind="ExternalInput") for t in "GL"}
    wm = {t: nc.dram_tensor(f"wm_{t}", [H, Z], F32R, kind="ExternalInput")
          for t in "GL"}
    bn_sc = {t: nc.dram_tensor(f"bn_sc_{t}", [H, 1], F32, kind="ExternalInput")
             for t in "GL"}
    bn_sh = {t: nc.dram_tensor(f"bn_sh_{t}", [H, 1], F32, kind="ExternalInput")
             for t in "GL"}
    zbias = nc.dram_tensor("zbias", [Z, 1], F32, kind="ExternalInput")
    ident = nc.dram_tensor("ident", [P, P], F32R,
                           kind="ExternalInput")  # I/N
    out_t = nc.dram_tensor("out_t", [Z, R], F32, kind="ExternalOutput")

    RG = [list(range(NCORES))]

    def grp_ap(dram_ap, g, rows_per_grp):
        return dram_ap[g * rows_per_grp:(g + 1) * rows_per_grp, :].rearrange(
            "(b p) d -> p b d", p=P)

    with tile.TileContext(nc) as tc:
        with (
            tc.tile_pool(name="atp", bufs=2 * KG) as atp,
            tc.tile_pool(name="chainp", bufs=2 * KG) as chainp,
            tc.tile_pool(name="wp", bufs=2) as wp,
            tc.tile_pool(name="ysp", bufs=3) as ysp,
            tc.tile_pool(name="h1tp", bufs=4) as h1tp,
            tc.tile_pool(name="smallp", bufs=2) as smallp,
            tc.tile_pool(name="hop_ps", bufs=5, space="PSUM") as hop_ps,
            tc.tile_pool(name="tp_ps", bufs=2, space="PSUM") as tp_ps,
            tc.tile_pool(name="acc2_ps", bufs=1, space="PSUM") as acc2_ps,
            tc.tile_pool(name="agin", bufs=2, space="DRAM") as agin,
            tc.tile_pool(name="agout", bufs=2, space="DRAM") as agout,
        ):
            for rep in range(T):
                dma_rr = [0]

                def dma(out_ap, in_ap):
                    # alternate the two HWDGE rings (SP / ACT) for parallelism
                    eng = (nc.sync, nc.scalar)[dma_rr[0] % 2]
                    dma_rr[0] += 1
                    eng.dma_start(out_ap, in_ap)

                ident_t = smallp.tile([P, P], F32R, name="ident", tag="ident")
                dma(ident_t[:], ident[:])
                zbias_t = smallp.tile([Z, 1], F32, name="zbias", tag="zb")
                dma(zbias_t[:], zbias[:])

                state = {}
                ACC2_TOTAL = 2 * (HT + KT // 2 + HT)

                def acc2_mm(lhsT, rhs, perf_mode=None):
                    nc.tensor.matmul(state["acc2"][:], lhsT, rhs,
                                     start=(state["n"] == 0),
                                     stop=(state["n"] == ACC2_TOTAL - 1),
                                     perf_mode=perf_mode)
                    state["n"] += 1

                def transpose_fp8(dst_ap, src_ap, name):
                    # dst = (src / N).T via PE matmul against I/N; DVE copy
                    # converts the f32 PSUM result to the fp8 gather shard.
                    tp = tp_ps.tile([P, P], F32, name=name, tag="tp")
                    nc.tensor.matmul(tp[:], src_ap, ident_t[:],
                                     start=True, stop=True)
                    nc.vector.tensor_copy(dst_ap, tp[:])

                def allgather(shard_big, width, tag, branch_tag):
                    bounce_in = agin.tile([R, width], FP8, name=f"agi_{tag}",
                                          tag="agin")
                    dma(bounce_in[:].rearrange("(b p) d -> p b d", p=P),
                        shard_big[:].rearrange("p (b d) -> p b d", b=MT))
                    bounce_out = agout.tile([N, width], FP8, name=f"ago_{tag}",
                                            tag="agout", addr_space="Shared")
                    nc.gpsimd.collective_compute(
                        "AllGather", mybir.AluOpType.bypass, replica_groups=RG,
                        ins=[bounce_in.opt()], outs=[bounce_out.opt()])
                    tiles = []
                    for g in range(KG):
                        t = chainp.tile([P, GRP, width], FP8,
                                        name=f"s_{tag}_{g}",
                                        tag=f"chain{branch_tag}")
                        dma(t[:], grp_ap(bounce_out, g, GRP * P))
                        tiles.append(t)
                    return tiles

                def hop_drmm(ps_list, s_tiles, at_t, width, stop_at_end):
                    # ps_list: one [P, R] PSUM per 128-wide feature slice.
                    ndt = width // P
                    for u in range(KT // 2):
                        g, b0 = u // 2, 2 * (u % 2)
                        rhs = at_t[g][:, b0:b0 + 2, :]
                        for dt in range(ndt):
                            nc.tensor.matmul(
                                ps_list[dt][:],
                                s_tiles[g][:, b0:b0 + 2, dt * P:(dt + 1) * P],
                                rhs, start=False,
                                stop=(stop_at_end and u == KT // 2 - 1),
                                perf_mode=DR)

                def to_shard_and_gather(ps_list, width, tag, branch_tag):
                    # PSUM stages -> SBUF f32r -> fp8 node-partition shard
                    # (transposed + descaled by 1/N on PE) -> AllGather.
                    ndt = width // P
                    st = ysp.tile([P, ndt * R], F32R, name=f"st_{tag}",
                                  tag=f"ys{branch_tag}")
                    for dt in range(ndt):
                        nc.vector.tensor_copy(st[:, dt * R:(dt + 1) * R],
                                              ps_list[dt][:])
                    shard = ysp.tile([P, MT * width], FP8, name=f"sh_{tag}",
                                     tag=f"ys{branch_tag}")
                    for m in range(MT):
                        for dt in range(ndt):
                            transpose_fp8(
                                shard[:, m * width + dt * P:
                                      m * width + (dt + 1) * P],
                                st[:, dt * R + m * P:dt * R + (m + 1) * P],
                                f"tp_{tag}_{m}_{dt}")
                    return allgather(shard, width, tag, branch_tag)

                def branch(tag):
                    w1_t = wp.tile([P, (KHOPS + 1) * DT1 * H], F32R,
                                   name=f"w1{tag}", tag="w1")
                    dma(w1_t[:].rearrange("p (b d) -> p b d", p=P, d=H),
                        w1[tag][:].rearrange("(b p) d -> p b d", p=P))
                    at_t = []
                    for g in range(KG):
                        t = atp.tile([P, GRP, R], FP8, name=f"at{tag}_{g}",
                                     tag=f"at{tag}")
                        dma(t[:], grp_ap(at[tag], g, GRP * P))
                        at_t.append(t)
                    w2_t = wp.tile([P, (KHOPS + 1) * HT * Z], F32R,
                                   name=f"w2{tag}", tag="w2")
                    dma(w2_t[:].rearrange("p (b d) -> p b d", p=P, d=Z),
                        w2[tag][:].rearrange("(b p) d -> p b d", p=P))
                    wm_t = wp.tile([P, HT * Z], F32R, name=f"wm{tag}",
                                   tag="wm")
                    dma(wm_t[:].rearrange("p (b d) -> p b d", p=P, d=Z),
                        wm[tag][:].rearrange("(b p) d -> p b d", p=P))
                    bn_sc_t = smallp.tile([P, HT], F32, name=f"bnsc{tag}",
                                          tag="bn1")
                    dma(bn_sc_t[:].rearrange("p (b d) -> p b d", p=P, d=1),
                        bn_sc[tag][:].rearrange("(b p) d -> p b d", p=P))
                    bn_sh_t = smallp.tile([P, HT], F32, name=f"bnsh{tag}",
                                          tag="bn2")
                    dma(bn_sh_t[:].rearrange("p (b d) -> p b d", p=P, d=1),
                        bn_sh[tag][:].rearrange("(b p) d -> p b d", p=P))
                    xt_t = state["xt_t"]
                    yield

                    def w1_slice(khop, dt, hf):
                        base = (khop * DT1 + dt) * H + hf * P
                        return w1_t[:, base:base + P]

                    def w2_slice(khop, dt):
                        base = (khop * HT + dt) * Z
                        return w2_t[:, base:base + Z]

                    # ---- conv1 Horner: s_k = u_k + A s_{k+1}, k = K..1 ----
                    s_tiles = None
                    for k in range(KHOPS, 0, -1):
                        ps = [hop_ps.tile([P, R], F32, name=f"s{tag}{k}_{hf}",
                                          tag="hop") for hf in range(HT)]
                        last_u = (k == KHOPS)
                        for hf in range(HT):
                            for dt in range(DT1):
                                nc.tensor.matmul(
                                    ps[hf][:], w1_slice(k, dt, hf),
                                    xt_t[:, dt * R:(dt + 1) * R],
                                    start=(dt == 0),
                                    stop=(last_u and dt == DT1 - 1))
                        yield
                        if k < KHOPS:
                            hop_drmm(ps, s_tiles, at_t, H, True)
                        s_tiles = to_shard_and_gather(ps, H, f"{tag}1_{k}",
                                                      tag)
                        yield

                    # ---- h = relu(BN(u_0 + A s_1)) ----
                    acc1 = [hop_ps.tile([P, R], F32, name=f"acc1{tag}_{hf}",
                                        tag="hop") for hf in range(HT)]
                    for hf in range(HT):
                        for dt in range(DT1):
                            nc.tensor.matmul(
                                acc1[hf][:], w1_slice(0, dt, hf),
                                xt_t[:, dt * R:(dt + 1) * R],
                                start=(dt == 0), stop=False)
                    yield
                    hop_drmm(acc1, s_tiles, at_t, H, True)
                    h1t = []
                    for hf in range(HT):
                        t = h1tp.tile([P, R], F32R, name=f"h1t{tag}_{hf}",
                                      tag="h1t")
                        nc.scalar.activation(t[:], acc1[hf][:], AF.Relu,
                                             bias=bn_sh_t[:, hf:hf + 1],
                                             scale=bn_sc_t[:, hf:hf + 1])
                        h1t.append(t)
                    yield

                    # ---- conv2 Horner: s'_k = v_k + A s'_{k+1}, k = K..1 ----
                    s_tiles = None
                    for k in range(KHOPS, 0, -1):
                        ps = [hop_ps.tile([P, R], F32, name=f"z{tag}{k}",
                                          tag="hop")]
                        last_v = (k == KHOPS)
                        for dt in range(HT):
                            nc.tensor.matmul(
                                ps[0][:], w2_slice(k, dt), h1t[dt][:],
                                start=(dt == 0),
                                stop=(last_v and dt == HT - 1))
                        yield
                        if k < KHOPS:
                            hop_drmm(ps, s_tiles, at_t, Z, True)
                        s_tiles = to_shard_and_gather(ps, Z, f"{tag}2_{k}",
                                                      tag)
                        yield

                    # ---- z = v_0 + A s'_1 + h @ (N Wm), into shared acc2 ----
                    for dt in range(HT):
                        acc2_mm(w2_slice(0, dt), h1t[dt][:])
                    for u in range(KT // 2):
                        g, b0 = u // 2, 2 * (u % 2)
                        acc2_mm(s_tiles[g][:, b0:b0 + 2, :],
                                at_t[g][:, b0:b0 + 2, :], perf_mode=DR)
                    for dt in range(HT):
                        acc2_mm(wm_t[:, dt * Z:(dt + 1) * Z], h1t[dt][:])

                state["acc2"] = acc2_ps.tile([P, R], F32, name="acc2",
                                             tag="acc2")
                state["n"] = 0
                xt_t = ysp.tile([P, DT1 * R], F32R, name="xt", tag="xt",
                                bufs=2)
                dma(xt_t[:].rearrange("p (b d) -> p b d", b=DT1),
                    xt_sh[:].rearrange("(b p) d -> p b d", p=P))
                state["xt_t"] = xt_t

                gens = [branch("G"), branch("L")]
                done = [False, False]
                while not all(done):
                    for i, g in enumerate(gens):
                        if not done[i]:
                            try:
                                next(g)
                            except StopIteration:
                                done[i] = True

                out_sb = ysp.tile([Z, R], F32, name="out_sb", tag="out")
                nc.scalar.activation(out_sb[:], state["acc2"][:], AF.Identity,
                                     bias=zbias_t[:], scale=1.0 / N)
                dma(out_t[:], out_sb[:])

    nc.compile()
    return nc


def _make_in_maps(inputs):
    import ml_dtypes
    f8 = ml_dtypes.float8_e4m3
    x = np.asarray(inputs["x"], np.float32)
    at_full = {t: np.ascontiguousarray(
        (np.asarray(inputs[f"A_{t}"], np.float32).T * N).astype(f8))
        for t in "GL"}
    prep = {}
    for t in "GL":
        g = np.asarray(inputs[f"gamma_{t}"], np.float32)
        b = np.asarray(inputs[f"beta_{t}"], np.float32)
        mu = np.asarray(inputs[f"mean_{t}"], np.float32)
        v = np.asarray(inputs[f"var_{t}"], np.float32)
        b1 = np.asarray(inputs[f"b1_{t}"], np.float32)
        sc = g / np.sqrt(v + EPS)
        sh = (b1 - mu) * sc + b
        # acc1 PSUM carries N * conv1 — fold the descale into the BN scale
        prep[f"bn_sc_{t}"] = np.ascontiguousarray((sc / N).reshape(H, 1))
        prep[f"bn_sh_{t}"] = np.ascontiguousarray(sh.reshape(H, 1))
        prep[f"w1_{t}"] = np.ascontiguousarray(
            np.asarray(inputs[f"W1_{t}"], np.float32) * N)
        prep[f"w2_{t}"] = np.ascontiguousarray(
            np.asarray(inputs[f"W2_{t}"], np.float32) * N)
        prep[f"wm_{t}"] = np.ascontiguousarray(
            np.asarray(inputs[f"Wm_{t}"], np.float32) * N)
    zb = sum(np.asarray(inputs[f"b2_{t}"], np.float32) +
             np.asarray(inputs[f"bm_{t}"], np.float32) for t in "GL")
    prep["zbias"] = np.ascontiguousarray(zb.reshape(Z, 1))
    prep["ident"] = np.ascontiguousarray(np.eye(P, dtype=np.float32) / N)
    in_maps = []
    for c in range(NCORES):
        sl = slice(c * R, (c + 1) * R)
        m = dict(prep)
        m["xt_sh"] = np.ascontiguousarray(x[sl].T)
        for t in "GL":
            m[f"at_{t}"] = np.ascontiguousarray(at_full[t][:, sl])
        in_maps.append(m)
    return in_maps


def _get_nc():
    if "nc" not in _CACHE:
        _CACHE["nc"] = _build()
    return _CACHE["nc"]


def kernel(**inputs) -> np.ndarray:
    from concourse.bass_utils import run_bass_kernel_spmd

    nc = _get_nc()
    in_maps = _make_in_maps(inputs)
    res = run_bass_kernel_spmd(nc, in_maps, list(range(NCORES)))
    out = np.empty((N, Z), np.float32)
    for c in range(NCORES):
        out[c * R:(c + 1) * R, :] = res.results[c]["out_t"].T
    return out
